# revision 21
# baseline (speedup 1.0000x reference)
"""Multi-head attention (B=2, S=2048, D=1024, H=16) on 8 Trainium2 NeuronCores.

Sharding: 2-way data parallel over batch x 4-way tensor parallel over heads.
Core c handles batch c//4 and heads [4*(c%4), 4*(c%4)+4).  Each core computes
its 4 heads' attention and a partial output projection; the host sums the 4
partials per batch element (the bias bo is only added by the g==0 cores).

Host-side prep passes activations and weights pre-transposed so every matmul
contracts over the SBUF partition dimension with contiguous DMA loads.

v3 notes:
- bf16 storage/IO end-to-end: halves HBM traffic and enables fast weight
  load on the PE.  (measured rel err ~9e-3 vs the 2e-2 gate)
- x loads batched to one 1 MB DMA per quarter.
- projection work for later rounds is emitted through a fine-grained filler
  queue: ~2 matmuls are dropped into each attention block's PE stall window
  (between the score matmuls and the exp-dependent AV matmuls), so the PE
  does projections during ACT-bound slack without delaying the exp stream.
- output stores ride the scalar (HWDGE) queue and SBUF->SBUF normalize
  moves ride the gpsimd (SWDGE) queue; the sync queue carries only the
  x-activation stream.
"""

import os
from contextlib import ExitStack

import numpy as np

import concourse.mybir as mybir
import concourse.tile as tile
from concourse import bacc
from concourse import bass_utils
from concourse._compat import with_exitstack

F32 = mybir.dt.float32
F32R = mybir.dt.float32r
BF16 = mybir.dt.bfloat16

# "f32r": fp32 storage, float32r matmuls.  "bf16": bf16 storage + matmuls.
DT_MODE = "bf16"
ABL = os.environ.get("ABL", "")  # timing ablations: noexp | nodep | nodma

D_MODEL = 1024
N_HEAD = 16
DK = 64
B = 2
S = 2048
N_CORES = 8
HPC = 4          # heads per core
DPC = HPC * DK   # 256 output dims per core
KC = D_MODEL // 128   # 8 contraction chunks of 128
SQ = 512         # sequence quarter
NSQ = S // SQ    # 4
NJB = S // 128   # 16 key blocks
NSB = S // 128   # 16 query/row blocks

if DT_MODE == "bf16":
    import ml_dtypes
    SB_DT = BF16
    IO_NP = ml_dtypes.bfloat16
else:
    SB_DT = F32R
    IO_NP = np.float32

ATT_DT = SB_DT
OUT_DT = BF16 if DT_MODE == "bf16" else F32


@with_exitstack
def build_mha(ctx: ExitStack, tc, ins, out_ap, loop_n=None):
    """Emit the per-core kernel.  loop_n wraps the whole compute body in a
    hardware For_i loop (used only for timing measurement)."""
    nc = tc.nc
    P = 128
    Exp = mybir.ActivationFunctionType.Exp
    Add = mybir.AluOpType.add

    xq = ins["xq_t"].rearrange("(kc p) s -> p kc s", p=P)
    xk = ins["xk_t"].rearrange("(kc p) s -> p kc s", p=P)
    xv = ins["xv_t"].rearrange("(kc p) s -> p kc s", p=P)
    out = out_ap.rearrange("(sb p) n -> p sb n", p=P)

    ec = ctx.enter_context
    cpool = ec(tc.tile_pool(name="consts", bufs=1))
    # bufs=7: the head requests 5 slots (K0,Q0,V0,K1,V1) and the 7 queued
    # quarters request 7 more; with 7 slots every push-time DMA reuses a slot
    # whose readers were already emitted in the head (WAR order correct), and
    # the next For_i iteration's head reuses slots consumed by this
    # iteration's filler.
    xpool = ec(tc.tile_pool(name="xs", bufs=7))
    qkpool = ec(tc.tile_pool(name="qk", bufs=1))
    vpool = ec(tc.tile_pool(name="vh", bufs=1))
    ptpool = ec(tc.tile_pool(name="pt", bufs=8))
    apool = ec(tc.tile_pool(name="attn", bufs=1))
    opool = ec(tc.tile_pool(name="outs", bufs=3))
    npool = ec(tc.tile_pool(name="nrm", bufs=2))
    accpool = ec(tc.tile_pool(name="acc", bufs=1))
    pp_ps = ec(tc.tile_pool(name="proj_ps", bufs=1, space="PSUM"))
    sc_ps = ec(tc.tile_pool(name="score_ps", bufs=2, space="PSUM"))
    at_ps = ec(tc.tile_pool(name="att_ps", bufs=2, space="PSUM"))

    # --- constants (outside the timing loop) ---
    wq_sb = cpool.tile([P, KC, DPC], SB_DT, tag="wq")
    wk_sb = cpool.tile([P, KC, DPC], SB_DT, tag="wk")
    wv_sb = cpool.tile([P, KC, DPC], SB_DT, tag="wv")
    wo_sb = cpool.tile([P, 2, D_MODEL], SB_DT, tag="wo")
    nc.scalar.dma_start(wk_sb[:], ins["wk_t"].rearrange("(kc p) m -> p kc m", p=P))
    nc.scalar.dma_start(wv_sb[:], ins["wv_t"].rearrange("(kc p) m -> p kc m", p=P))
    nc.scalar.dma_start(wq_sb[:], ins["wq_t"].rearrange("(kc p) m -> p kc m", p=P))
    nc.gpsimd.dma_start(wo_sb[:], ins["wo_t"].rearrange("(c p) n -> p c n", p=P))
    bq_sb = cpool.tile([P, 2], F32, tag="bq")
    bk_sb = cpool.tile([P, 2], F32, tag="bk")
    bv_sb = cpool.tile([P, DPC], F32, tag="bv")
    bo_sb = cpool.tile([P, D_MODEL], F32, tag="bo")
    nc.gpsimd.dma_start(bq_sb[:], ins["bq_p"][:])
    nc.gpsimd.dma_start(bk_sb[:], ins["bk_p"][:])
    nc.gpsimd.dma_start(bv_sb[:], ins["bv_b"][:])
    nc.gpsimd.dma_start(bo_sb[:], ins["bo_b"][:])

    # --- persistent activations ---
    qh_sb = qkpool.tile([P, 2, S], ATT_DT, tag="qh")   # [dk%128, head_pair, s]
    kh_sb = qkpool.tile([P, 2, S], ATT_DT, tag="kh")
    vh_sb = vpool.tile([P, NJB, HPC, DK + 1], ATT_DT, tag="vh")  # + ones col
    at_sb = apool.tile([P, 2, S], SB_DT, tag="at")    # attn out, transposed

    # memset f32 then broadcast-copy (walrus can't memset float32r)
    ones1 = cpool.tile([P, 1], F32, tag="ones1")
    nc.vector.memset(ones1[:], 1.0)
    nc.vector.tensor_copy(
        vh_sb[:, :, :, DK : DK + 1],
        ones1[:, None, None, :].to_broadcast((P, NJB, HPC, 1)),
    )
    if ABL == "noexp":
        ptc = cpool.tile([P, 1024], ATT_DT, tag="ptc")
        nc.vector.tensor_copy(ptc[:], ones1[:, 0:1].to_broadcast((P, 1024)))
    if ABL == "nodep":
        scc = cpool.tile([P, 1024], F32, tag="scc")
        nc.vector.tensor_copy(scc[:], ones1[:, 0:1].to_broadcast((P, 1024)))
    if ABL == "nodma":
        xconst = {}
        for nm, ap in (("q", xq), ("k", xk), ("v", xv)):
            for sq in range(NSQ):
                xc = cpool.tile([P, KC, SQ], SB_DT, tag=f"xc{nm}{sq}",
                                name=f"xc{nm}{sq}")
                nc.sync.dma_start(xc[:], ap[:, :, sq * SQ : (sq + 1) * SQ])
                xconst[(nm, sq)] = xc

    # fine-grained projection filler queue: (key, thunk) pairs
    fill_q = []

    def _fill(k):
        for _ in range(min(k, len(fill_q))):
            fill_q.pop(0)[1]()

    def _fill_until(key):
        """Correctness barrier: drain every thunk up to and including the
        last one tagged `key` (no-op when pacing already drained them)."""
        if not any(k == key for k, _ in fill_q):
            return
        last = max(i for i, (k, _) in enumerate(fill_q) if k == key)
        for _ in range(last + 1):
            fill_q.pop(0)[1]()

    def _flush_fill():
        while fill_q:
            fill_q.pop(0)[1]()

    def _load_quarter(nm, x_ap, sq):
        """One 1MB DMA for a full [d_model, SQ] activation quarter."""
        if ABL == "nodma":
            return xconst[(nm, sq)]
        xt = xpool.tile([P, KC, SQ], SB_DT, tag="xt")
        nc.sync.dma_start(xt[:], x_ap[:, :, sq * SQ : (sq + 1) * SQ])
        return xt

    def _qk_quarter(nm, x_ap, w_sb, b_sb, dst, sq, queue=False):
        xt = _load_quarter(nm, x_ap, sq)
        ps = pp_ps.tile([P, 1024], F32, tag="pp")

        def mk_mm(kc):
            def mm():
                nc.tensor.matmul(
                    ps[:, 0:512], w_sb[:, kc, 0:128], xt[:, kc, :],
                    start=(kc == 0), stop=(kc == KC - 1),
                )
            return mm

        def mk_mm2(kc):
            def mm():
                nc.tensor.matmul(
                    ps[:, 512:1024], w_sb[:, kc, 128:256], xt[:, kc, :],
                    start=(kc == 0), stop=(kc == KC - 1),
                )
            return mm

        def b0():
            nc.vector.tensor_scalar_add(
                dst[:, 0, sq * SQ : (sq + 1) * SQ], ps[:, 0:512], b_sb[:, 0:1]
            )

        def b1():
            nc.vector.tensor_scalar_add(
                dst[:, 1, sq * SQ : (sq + 1) * SQ], ps[:, 512:1024], b_sb[:, 1:2]
            )

        thunks = []
        for kc in range(KC):
            thunks += [mk_mm(kc), mk_mm2(kc)]
        thunks += [b0, b1]
        if queue:
            fill_q.extend((f"{nm}{sq}", th) for th in thunks)
        else:
            for th in thunks:
                th()

    def _v_quarter(sq, queue=False):
        # natural layout [s, dv]; row-blocks sharing a PSUM bank run their
        # accumulation groups sequentially over the pre-loaded quarter
        xt = _load_quarter("v", xv, sq)
        ps = pp_ps.tile([P, 1024], F32, tag="pp")

        def mk_mm(sbi, kc):
            def mm():
                nc.tensor.matmul(
                    ps[:, sbi * 256 : (sbi + 1) * 256],
                    xt[:, kc, sbi * 128 : (sbi + 1) * 128],
                    wv_sb[:, kc, :],
                    start=(kc == 0), stop=(kc == KC - 1),
                )
            return mm

        def mk_bias(sbi):
            def bias():
                jb = sq * 4 + sbi
                nc.vector.tensor_tensor(
                    vh_sb[:, jb, :, 0:DK],
                    ps[:, sbi * 256 : (sbi + 1) * 256].rearrange(
                        "p (h d) -> p h d", h=HPC),
                    bv_sb[:].rearrange("p (h d) -> p h d", h=HPC),
                    Add,
                )
            return bias

        thunks = []
        for sbi in range(4):
            thunks += [mk_mm(sbi, kc) for kc in range(KC)]
            thunks.append(mk_bias(sbi))
        if queue:
            fill_q.extend((f"v{sq}", th) for th in thunks)
        else:
            for th in thunks:
                th()

    # attention partial accumulators, one per (head, query-quarter);
    # row 64 carries the running sum(exp) for the softmax denominator
    acc_sb = [
        [accpool.tile([65, 512], F32, tag=f"acc{i5}_{h}", name=f"acc{i5}_{h}") for h in range(HPC)]
        for i5 in range(NSQ)
    ]

    def _attn_block(i5, t, kq):
        """8 key-blocks (one key half) of attention for head pair t, query
        quarter i5."""
        # correctness barriers: the projections this group consumes must be
        # emitted before its reads (no-ops when filler pacing kept up)
        _fill_until(f"q{i5}")
        for sq in (2 * kq, 2 * kq + 1):
            _fill_until(f"k{sq}")
            _fill_until(f"v{sq}")
        i_sl = slice(i5 * SQ, (i5 + 1) * SQ)
        att_e = at_ps.tile([P, 512], F32, tag="att")
        att_o = at_ps.tile([P, 512], F32, tag="att")
        pts = []
        jbs = range(kq * 8, kq * 8 + 8)
        for n, jb in enumerate(jbs):
            sc = sc_ps.tile([P, 1024], F32, tag="sc")
            j_sl = slice(jb * 128, (jb + 1) * 128)
            nc.tensor.matmul(
                sc[:, 0:512], kh_sb[0:64, t, j_sl], qh_sb[0:64, t, i_sl],
                start=True, stop=True,
            )
            nc.tensor.matmul(
                sc[:, 512:1024], kh_sb[64:128, t, j_sl],
                qh_sb[64:128, t, i_sl], start=True, stop=True,
            )
            if ABL == "noexp":
                pt = ptc
            else:
                pt = ptpool.tile([P, 1024], ATT_DT, tag="pt")
                src = scc if ABL == "nodep" else sc
                nc.scalar.activation(pt[:], src[:], Exp, scale=1.0 / np.sqrt(DK))
            pts.append(pt)
            _fill(2)
            if n > 0:
                ptp = pts[n - 1]
                nc.tensor.matmul(
                    att_e[0:65, :], vh_sb[:, jb - 1, 2 * t, :],
                    ptp[:, 0:512], start=(n - 1 == 0), stop=False,
                )
                nc.tensor.matmul(
                    att_o[0:65, :], vh_sb[:, jb - 1, 2 * t + 1, :],
                    ptp[:, 512:1024], start=(n - 1 == 0), stop=False,
                )
        jb_last = kq * 8 + 7
        nc.tensor.matmul(
            att_e[0:65, :], vh_sb[:, jb_last, 2 * t, :],
            pts[-1][:, 0:512], start=False, stop=True,
        )
        nc.tensor.matmul(
            att_o[0:65, :], vh_sb[:, jb_last, 2 * t + 1, :],
            pts[-1][:, 512:1024], start=False, stop=True,
        )
        for h, aps in ((2 * t, att_e), (2 * t + 1, att_o)):
            acc = acc_sb[i5][h]
            if kq == 0:
                nc.vector.tensor_copy(acc[:], aps[0:65, :])
            else:
                nc.vector.tensor_tensor(acc[:], acc[:], aps[0:65, :], Add)
        # group-end slack is larger: ACT still owes this group's last exps
        _fill(6)

    def _normalize(i5):
        i_sl = slice(i5 * SQ, (i5 + 1) * SQ)
        for h in range(HPC):
            acc = acc_sb[i5][h]
            t = h // 2
            rc = npool.tile([1, 512], F32, tag="rc")
            nc.vector.reciprocal(rc[:], acc[64:65, :])
            bc = npool.tile([64, 512], F32, tag="bc")
            nc.gpsimd.partition_broadcast(bc[:], rc[:])
            if h % 2 == 0:
                nc.vector.tensor_mul(at_sb[0:64, t, i_sl], acc[0:64, :], bc[:])
            else:
                tm = npool.tile([64, 512], SB_DT, tag="tm")
                nc.vector.tensor_mul(tm[:], acc[0:64, :], bc[:])
                nc.gpsimd.dma_start(at_sb[64:128, t, i_sl], tm[:])

    def _final(i5):
        for sbi in range(4):
            sb = i5 * 4 + sbi
            s_sl = slice(sb * 128, (sb + 1) * 128)
            po = pp_ps.tile([P, 1024], F32, tag="pp")
            for c in range(2):
                nc.tensor.matmul(
                    po[:, 0:512], at_sb[:, c, s_sl], wo_sb[:, c, 0:512],
                    start=(c == 0), stop=(c == 1),
                )
                nc.tensor.matmul(
                    po[:, 512:1024], at_sb[:, c, s_sl], wo_sb[:, c, 512:1024],
                    start=(c == 0), stop=(c == 1),
                )
            ot = opool.tile([P, 1024], OUT_DT, tag="ot")
            nc.vector.tensor_tensor(ot[:], po[:], bo_sb[:], Add)
            nc.scalar.dma_start(out[:, sb, :], ot[:])

    def _compute():
        # Two key-half rounds: the head projects K/V half 0 and Q quarter 0
        # directly (hidden under the previous iteration's tail in the For_i
        # steady state); everything else drains through the filler queue
        # inside the attention blocks' PE stall windows.
        _qk_quarter("k", xk, wk_sb, bk_sb, kh_sb, 0)
        _qk_quarter("q", xq, wq_sb, bq_sb, qh_sb, 0)
        _v_quarter(0)
        _qk_quarter("k", xk, wk_sb, bk_sb, kh_sb, 1)
        _v_quarter(1)
        # push order fixes both filler deadlines (q quarters first) and the
        # xpool slot-reuse pattern (exactly 7 queued quarters on 7 bufs)
        for sq in range(1, NSQ):
            _qk_quarter("q", xq, wq_sb, bq_sb, qh_sb, sq, queue=True)
        for sq in (2, 3):
            _qk_quarter("k", xk, wk_sb, bk_sb, kh_sb, sq, queue=True)
            _v_quarter(sq, queue=True)
        for kq in range(2):
            for i5 in range(NSQ):
                for t in range(2):
                    _attn_block(i5, t, kq)
                if kq == 1:
                    _normalize(i5)
                    _final(i5)
        _flush_fill()

    if loop_n is not None and loop_n > 1:
        with tc.For_i(0, loop_n, 1):
            _compute()
    else:
        _compute()


def shard_inputs(q, k, v, Wq, bq, Wk, bk, Wv, bv, Wo, bo):
    """Build the 8 per-core input maps from the full inputs."""
    def prep(a):
        return np.ascontiguousarray(np.asarray(a, np.float32)).astype(IO_NP)

    # x transposes are shared by the 4 cores of a batch group: convert once
    xt_cache = [
        {n: prep(np.asarray(x)[b].T) for n, x in (("xq_t", q), ("xk_t", k), ("xv_t", v))}
        for b in range(B)
    ]

    in_maps = []
    for c in range(N_CORES):
        b, g = divmod(c, 4)
        hs = slice(g * DPC, (g + 1) * DPC)
        bo_b = (
            np.broadcast_to(np.asarray(bo, np.float32), (128, D_MODEL))
            if g == 0
            else np.zeros((128, D_MODEL), np.float32)
        )
        in_maps.append({
            **xt_cache[b],
            "wq_t": prep(np.asarray(Wq)[hs, :].T),
            "wk_t": prep(np.asarray(Wk)[hs, :].T),
            "wv_t": prep(np.asarray(Wv)[hs, :].T),
            "wo_t": prep(np.asarray(Wo)[:, hs].T),
            "bq_p": np.ascontiguousarray(
                np.asarray(bq, np.float32)[hs].reshape(2, 128).T),
            "bk_p": np.ascontiguousarray(
                np.asarray(bk, np.float32)[hs].reshape(2, 128).T),
            "bv_b": np.ascontiguousarray(
                np.broadcast_to(np.asarray(bv, np.float32)[hs], (128, DPC))),
            "bo_b": np.ascontiguousarray(bo_b),
        })
    return in_maps


_NC = None


def build_nc(loop_n=None):
    nc = bacc.Bacc(
        "TRN2",
        target_bir_lowering=False,
        debug=False,
        enable_asserts=False,
        num_devices=N_CORES,
    )
    ins = {}
    for name in ("xq_t", "xk_t", "xv_t"):
        ins[name] = nc.dram_tensor(
            name, [D_MODEL, S], SB_DT, kind="ExternalInput").ap()
    for name in ("wq_t", "wk_t", "wv_t"):
        ins[name] = nc.dram_tensor(
            name, [D_MODEL, DPC], SB_DT, kind="ExternalInput").ap()
    ins["wo_t"] = nc.dram_tensor(
        "wo_t", [DPC, D_MODEL], SB_DT, kind="ExternalInput").ap()
    ins["bq_p"] = nc.dram_tensor("bq_p", [128, 2], F32, kind="ExternalInput").ap()
    ins["bk_p"] = nc.dram_tensor("bk_p", [128, 2], F32, kind="ExternalInput").ap()
    ins["bv_b"] = nc.dram_tensor("bv_b", [128, DPC], F32, kind="ExternalInput").ap()
    ins["bo_b"] = nc.dram_tensor(
        "bo_b", [128, D_MODEL], F32, kind="ExternalInput").ap()
    out_ap = nc.dram_tensor("out", [S, D_MODEL], OUT_DT, kind="ExternalOutput").ap()
    with tile.TileContext(nc) as tc:
        build_mha(tc, ins, out_ap, loop_n=loop_n)
    nc.compile()
    return nc


def _get_nc():
    global _NC
    if _NC is None:
        _NC = build_nc()
    return _NC


def run_sharded(inputs, trace=False):
    nc = _get_nc()
    in_maps = shard_inputs(**inputs)
    res = bass_utils.run_bass_kernel_spmd(
        nc, in_maps, core_ids=list(range(N_CORES)), trace=trace
    )
    acc = np.zeros((B, S, D_MODEL), np.float64)
    for c in range(N_CORES):
        acc[c // 4] += np.asarray(res.results[c]["out"]).astype(np.float64)
    return acc.astype(np.float32), res


def kernel(**inputs):
    out, _ = run_sharded(inputs, trace=False)
    return out


# revision 27
# speedup vs baseline: 1.7931x; 1.7931x over previous
"""Multi-head attention (B=2, S=2048, D=1024, H=16) on 8 Trainium2 NeuronCores.

Sharding: 2-way data parallel over batch x 4-way tensor parallel over heads.
Core c handles batch c//4 and heads [4*(c%4), 4*(c%4)+4).  Each core computes
its 4 heads' attention and a partial output projection; the host sums the 4
partials per batch element (the bias bo is only added by the g==0 cores).

Host-side prep passes activations and weights pre-transposed so every matmul
contracts over the SBUF partition dimension with contiguous DMA loads.

v3 notes:
- bf16 storage/IO end-to-end: halves HBM traffic and enables fast weight
  load on the PE.  (measured rel err ~9e-3 vs the 2e-2 gate)
- x loads batched to one 1 MB DMA per quarter.
- projection work for later rounds is emitted through a fine-grained filler
  queue: ~2 matmuls are dropped into each attention block's PE stall window
  (between the score matmuls and the exp-dependent AV matmuls), so the PE
  does projections during ACT-bound slack without delaying the exp stream.
- output stores ride the scalar (HWDGE) queue and SBUF->SBUF normalize
  moves ride the gpsimd (SWDGE) queue; the sync queue carries only the
  x-activation stream.
"""

import os
from contextlib import ExitStack

import numpy as np

import concourse.mybir as mybir
import concourse.tile as tile
from concourse import bacc
from concourse import bass_utils
from concourse._compat import with_exitstack

F32 = mybir.dt.float32
F32R = mybir.dt.float32r
BF16 = mybir.dt.bfloat16

# "f32r": fp32 storage, float32r matmuls.  "bf16": bf16 storage + matmuls.
DT_MODE = "bf16"
ABL = os.environ.get("ABL", "")  # timing ablations: noexp | nodep | nodma

D_MODEL = 1024
N_HEAD = 16
DK = 64
B = 2
S = 2048
N_CORES = 8
HPC = 4          # heads per core
DPC = HPC * DK   # 256 output dims per core
KC = D_MODEL // 128   # 8 contraction chunks of 128
SQ = 512         # sequence quarter
NSQ = S // SQ    # 4
NJB = S // 128   # 16 key blocks
NSB = S // 128   # 16 query/row blocks

if DT_MODE == "bf16":
    import ml_dtypes
    SB_DT = BF16
    IO_NP = ml_dtypes.bfloat16
else:
    SB_DT = F32R
    IO_NP = np.float32

ATT_DT = SB_DT
OUT_DT = BF16 if DT_MODE == "bf16" else F32


@with_exitstack
def build_mha(ctx: ExitStack, tc, ins, out_ap, loop_n=None):
    """Emit the per-core kernel.  loop_n wraps the whole compute body in a
    hardware For_i loop (used only for timing measurement)."""
    nc = tc.nc
    P = 128
    Exp = mybir.ActivationFunctionType.Exp
    Add = mybir.AluOpType.add

    xq = ins["xq_t"].rearrange("(kc p) s -> p kc s", p=P)
    xk = ins["xk_t"].rearrange("(kc p) s -> p kc s", p=P)
    xv = ins["xv_t"].rearrange("(kc p) s -> p kc s", p=P)
    out = out_ap.rearrange("(sb p) n -> p sb n", p=P)

    ec = ctx.enter_context
    cpool = ec(tc.tile_pool(name="consts", bufs=1))
    # bufs=5: with the head requesting 3 slots (K0,Q0,V0) and each round's
    # queued quarters requesting more, 5 slots guarantee every push-time DMA
    # reuses a slot whose readers were already emitted (WAR order correct).
    xpool = ec(tc.tile_pool(name="xs", bufs=5))
    qkpool = ec(tc.tile_pool(name="qk", bufs=1))
    vpool = ec(tc.tile_pool(name="vh", bufs=1))
    ptpool = ec(tc.tile_pool(name="pt", bufs=6))
    apool = ec(tc.tile_pool(name="attn", bufs=1))
    opool = ec(tc.tile_pool(name="outs", bufs=3))
    npool = ec(tc.tile_pool(name="nrm", bufs=4))
    accpool = ec(tc.tile_pool(name="acc", bufs=1))
    pp_ps = ec(tc.tile_pool(name="proj_ps", bufs=1, space="PSUM"))
    sc_ps = ec(tc.tile_pool(name="score_ps", bufs=2, space="PSUM"))
    at_ps = ec(tc.tile_pool(name="att_ps", bufs=2, space="PSUM"))

    # --- constants (outside the timing loop) ---
    wq_sb = cpool.tile([P, KC, DPC], SB_DT, tag="wq")
    wk_sb = cpool.tile([P, KC, DPC], SB_DT, tag="wk")
    wv_sb = cpool.tile([P, KC, DPC], SB_DT, tag="wv")
    wo_sb = cpool.tile([P, 2, D_MODEL], SB_DT, tag="wo")
    nc.scalar.dma_start(wk_sb[:], ins["wk_t"].rearrange("(kc p) m -> p kc m", p=P))
    nc.scalar.dma_start(wv_sb[:], ins["wv_t"].rearrange("(kc p) m -> p kc m", p=P))
    nc.scalar.dma_start(wq_sb[:], ins["wq_t"].rearrange("(kc p) m -> p kc m", p=P))
    nc.gpsimd.dma_start(wo_sb[:], ins["wo_t"].rearrange("(c p) n -> p c n", p=P))
    bq_sb = cpool.tile([P, 2], F32, tag="bq")
    bk_sb = cpool.tile([P, 2], F32, tag="bk")
    bv_sb = cpool.tile([P, DPC], F32, tag="bv")
    bo_sb = cpool.tile([P, D_MODEL], F32, tag="bo")
    nc.gpsimd.dma_start(bq_sb[:], ins["bq_p"][:])
    nc.gpsimd.dma_start(bk_sb[:], ins["bk_p"][:])
    nc.gpsimd.dma_start(bv_sb[:], ins["bv_b"][:])
    nc.gpsimd.dma_start(bo_sb[:], ins["bo_b"][:])

    # --- persistent activations ---
    qh_sb = qkpool.tile([P, 2, S], ATT_DT, tag="qh")   # [dk%128, head_pair, s]
    kh_sb = qkpool.tile([P, 2, S], ATT_DT, tag="kh")
    vh_sb = vpool.tile([P, NJB, HPC, DK + 1], ATT_DT, tag="vh")  # + ones col
    at_sb = apool.tile([P, 2, S], SB_DT, tag="at")    # attn out, transposed

    # memset f32 then broadcast-copy (walrus can't memset float32r)
    ones1 = cpool.tile([P, 1], F32, tag="ones1")
    nc.vector.memset(ones1[:], 1.0)
    nc.vector.tensor_copy(
        vh_sb[:, :, :, DK : DK + 1],
        ones1[:, None, None, :].to_broadcast((P, NJB, HPC, 1)),
    )
    if ABL == "noexp":
        ptc = cpool.tile([P, 1024], ATT_DT, tag="ptc")
        nc.vector.tensor_copy(ptc[:], ones1[:, 0:1].to_broadcast((P, 1024)))
    if ABL == "nodep":
        scc = cpool.tile([P, 1024], F32, tag="scc")
        nc.vector.tensor_copy(scc[:], ones1[:, 0:1].to_broadcast((P, 1024)))
    if ABL == "nodma":
        xconst = {}
        for nm, ap in (("q", xq), ("k", xk), ("v", xv)):
            for sq in range(NSQ):
                xc = cpool.tile([P, KC, SQ], SB_DT, tag=f"xc{nm}{sq}",
                                name=f"xc{nm}{sq}")
                nc.sync.dma_start(xc[:], ap[:, :, sq * SQ : (sq + 1) * SQ])
                xconst[(nm, sq)] = xc

    # fine-grained projection filler queue: (key, thunk) pairs
    fill_q = []

    def _fill(k):
        for _ in range(min(k, len(fill_q))):
            fill_q.pop(0)[1]()

    def _fill_until(key):
        """Correctness barrier: drain every thunk up to and including the
        last one tagged `key` (no-op when pacing already drained them)."""
        if not any(k == key for k, _ in fill_q):
            return
        last = max(i for i, (k, _) in enumerate(fill_q) if k == key)
        for _ in range(last + 1):
            fill_q.pop(0)[1]()

    def _flush_fill():
        while fill_q:
            fill_q.pop(0)[1]()

    def _load_quarter(nm, x_ap, sq):
        """One 1MB DMA for a full [d_model, SQ] activation quarter."""
        if ABL == "nodma":
            return xconst[(nm, sq)]
        xt = xpool.tile([P, KC, SQ], SB_DT, tag="xt")
        nc.sync.dma_start(xt[:], x_ap[:, :, sq * SQ : (sq + 1) * SQ])
        return xt

    def _qk_quarter(nm, x_ap, w_sb, b_sb, dst, sq, queue=False):
        xt = _load_quarter(nm, x_ap, sq)
        ps = pp_ps.tile([P, 1024], F32, tag="pp")

        def mk_mm(kc):
            def mm():
                nc.tensor.matmul(
                    ps[:, 0:512], w_sb[:, kc, 0:128], xt[:, kc, :],
                    start=(kc == 0), stop=(kc == KC - 1),
                )
            return mm

        def mk_mm2(kc):
            def mm():
                nc.tensor.matmul(
                    ps[:, 512:1024], w_sb[:, kc, 128:256], xt[:, kc, :],
                    start=(kc == 0), stop=(kc == KC - 1),
                )
            return mm

        def b0():
            nc.vector.tensor_scalar_add(
                dst[:, 0, sq * SQ : (sq + 1) * SQ], ps[:, 0:512], b_sb[:, 0:1]
            )

        def b1():
            nc.vector.tensor_scalar_add(
                dst[:, 1, sq * SQ : (sq + 1) * SQ], ps[:, 512:1024], b_sb[:, 1:2]
            )

        thunks = []
        for kc in range(KC):
            thunks += [mk_mm(kc), mk_mm2(kc)]
        thunks += [b0, b1]
        if queue:
            fill_q.extend((f"{nm}{sq}", th) for th in thunks)
        else:
            for th in thunks:
                th()

    def _v_quarter(sq, queue=False):
        # natural layout [s, dv]; row-blocks sharing a PSUM bank run their
        # accumulation groups sequentially over the pre-loaded quarter
        xt = _load_quarter("v", xv, sq)
        ps = pp_ps.tile([P, 1024], F32, tag="pp")

        def mk_mm(sbi, kc):
            def mm():
                nc.tensor.matmul(
                    ps[:, sbi * 256 : (sbi + 1) * 256],
                    xt[:, kc, sbi * 128 : (sbi + 1) * 128],
                    wv_sb[:, kc, :],
                    start=(kc == 0), stop=(kc == KC - 1),
                )
            return mm

        def mk_bias(sbi):
            def bias():
                jb = sq * 4 + sbi
                nc.vector.tensor_tensor(
                    vh_sb[:, jb, :, 0:DK],
                    ps[:, sbi * 256 : (sbi + 1) * 256].rearrange(
                        "p (h d) -> p h d", h=HPC),
                    bv_sb[:].rearrange("p (h d) -> p h d", h=HPC),
                    Add,
                )
            return bias

        thunks = []
        for sbi in range(4):
            thunks += [mk_mm(sbi, kc) for kc in range(KC)]
            thunks.append(mk_bias(sbi))
        if queue:
            fill_q.extend((f"v{sq}", th) for th in thunks)
        else:
            for th in thunks:
                th()

    # attention partial accumulators, one per (head, query-quarter);
    # row 64 carries the running sum(exp) for the softmax denominator
    acc_sb = [
        [accpool.tile([65, 512], F32, tag=f"acc{i5}_{h}", name=f"acc{i5}_{h}") for h in range(HPC)]
        for i5 in range(NSQ)
    ]

    def _attn_block(i5, t, jq):
        """4 key-blocks of attention for head pair t, query quarter i5."""
        # correctness barriers: the projections this group consumes must be
        # emitted before its reads (no-ops when filler pacing kept up)
        _fill_until(f"q{i5}")
        _fill_until(f"k{jq}")
        _fill_until(f"v{jq}")
        i_sl = slice(i5 * SQ, (i5 + 1) * SQ)
        att_e = at_ps.tile([P, 512], F32, tag="att")
        att_o = at_ps.tile([P, 512], F32, tag="att")
        pts = []
        jbs = range(jq * 4, jq * 4 + 4)
        for n, jb in enumerate(jbs):
            sc = sc_ps.tile([P, 1024], F32, tag="sc")
            j_sl = slice(jb * 128, (jb + 1) * 128)
            nc.tensor.matmul(
                sc[:, 0:512], kh_sb[0:64, t, j_sl], qh_sb[0:64, t, i_sl],
                start=True, stop=True,
            )
            nc.tensor.matmul(
                sc[:, 512:1024], kh_sb[64:128, t, j_sl],
                qh_sb[64:128, t, i_sl], start=True, stop=True,
            )
            if ABL == "noexp":
                pt = ptc
            else:
                pt = ptpool.tile([P, 1024], ATT_DT, tag="pt")
                src = scc if ABL == "nodep" else sc
                nc.scalar.activation(pt[:], src[:], Exp, scale=1.0 / np.sqrt(DK))
            pts.append(pt)
            _fill(2)
            if n > 0:
                ptp = pts[n - 1]
                nc.tensor.matmul(
                    att_e[0:65, :], vh_sb[:, jb - 1, 2 * t, :],
                    ptp[:, 0:512], start=(n - 1 == 0), stop=False,
                )
                nc.tensor.matmul(
                    att_o[0:65, :], vh_sb[:, jb - 1, 2 * t + 1, :],
                    ptp[:, 512:1024], start=(n - 1 == 0), stop=False,
                )
        jb_last = jq * 4 + 3
        nc.tensor.matmul(
            att_e[0:65, :], vh_sb[:, jb_last, 2 * t, :],
            pts[-1][:, 0:512], start=False, stop=True,
        )
        nc.tensor.matmul(
            att_o[0:65, :], vh_sb[:, jb_last, 2 * t + 1, :],
            pts[-1][:, 512:1024], start=False, stop=True,
        )
        for h, aps in ((2 * t, att_e), (2 * t + 1, att_o)):
            acc = acc_sb[i5][h]
            if jq == 0:
                nc.vector.tensor_copy(acc[:], aps[0:65, :])
            else:
                nc.vector.tensor_tensor(acc[:], acc[:], aps[0:65, :], Add)
        # group-end slack is larger: ACT still owes this group's last exps
        _fill(6)

    def _normalize(i5):
        i_sl = slice(i5 * SQ, (i5 + 1) * SQ)
        for h in range(HPC):
            acc = acc_sb[i5][h]
            t = h // 2
            rc = npool.tile([1, 512], F32, tag="rc")
            nc.vector.reciprocal(rc[:], acc[64:65, :])
            bc = npool.tile([64, 512], F32, tag="bc")
            nc.gpsimd.partition_broadcast(bc[:], rc[:])
            if h % 2 == 0:
                nc.vector.tensor_mul(at_sb[0:64, t, i_sl], acc[0:64, :], bc[:])
            else:
                tm = npool.tile([64, 512], SB_DT, tag="tm")
                nc.vector.tensor_mul(tm[:], acc[0:64, :], bc[:])
                nc.gpsimd.dma_start(at_sb[64:128, t, i_sl], tm[:])

    def _final(i5):
        for sbi in range(4):
            sb = i5 * 4 + sbi
            s_sl = slice(sb * 128, (sb + 1) * 128)
            po = pp_ps.tile([P, 1024], F32, tag="pp")
            for c in range(2):
                nc.tensor.matmul(
                    po[:, 0:512], at_sb[:, c, s_sl], wo_sb[:, c, 0:512],
                    start=(c == 0), stop=(c == 1),
                )
                nc.tensor.matmul(
                    po[:, 512:1024], at_sb[:, c, s_sl], wo_sb[:, c, 512:1024],
                    start=(c == 0), stop=(c == 1),
                )
            ot = opool.tile([P, 1024], OUT_DT, tag="ot")
            nc.vector.tensor_tensor(ot[:], po[:], bo_sb[:], Add)
            nc.scalar.dma_start(out[:, sb, :], ot[:])

    def _compute():
        # Stream key/value quarters: as soon as K/V quarter jq is projected,
        # all heads' attention over those 4 key blocks runs and accumulates
        # (value-weighted sums + sum-exp) into SBUF accumulators.  Later
        # projection quarters drain through the filler queue inside the
        # attention blocks' PE stall windows.
        _qk_quarter("k", xk, wk_sb, bk_sb, kh_sb, 0)
        _qk_quarter("q", xq, wq_sb, bq_sb, qh_sb, 0)
        _v_quarter(0)
        for jq in range(NSQ):
            if jq == 0:
                for sq in range(1, NSQ):
                    _qk_quarter("q", xq, wq_sb, bq_sb, qh_sb, sq, queue=True)
                _qk_quarter("k", xk, wk_sb, bk_sb, kh_sb, 1, queue=True)
                _v_quarter(1, queue=True)
            elif jq < NSQ - 1:
                _qk_quarter("k", xk, wk_sb, bk_sb, kh_sb, jq + 1, queue=True)
                _v_quarter(jq + 1, queue=True)
            for i5 in range(NSQ):
                for t in range(2):
                    _attn_block(i5, t, jq)
                if jq == NSQ - 1:
                    _normalize(i5)
                    _final(i5)
        _flush_fill()

    if loop_n is not None and loop_n > 1:
        with tc.For_i(0, loop_n, 1):
            _compute()
    else:
        _compute()


def shard_inputs(q, k, v, Wq, bq, Wk, bk, Wv, bv, Wo, bo):
    """Build the 8 per-core input maps from the full inputs."""
    def prep(a):
        return np.ascontiguousarray(np.asarray(a, np.float32)).astype(IO_NP)

    # x transposes are shared by the 4 cores of a batch group: convert once
    xt_cache = [
        {n: prep(np.asarray(x)[b].T) for n, x in (("xq_t", q), ("xk_t", k), ("xv_t", v))}
        for b in range(B)
    ]

    in_maps = []
    for c in range(N_CORES):
        b, g = divmod(c, 4)
        hs = slice(g * DPC, (g + 1) * DPC)
        bo_b = (
            np.broadcast_to(np.asarray(bo, np.float32), (128, D_MODEL))
            if g == 0
            else np.zeros((128, D_MODEL), np.float32)
        )
        in_maps.append({
            **xt_cache[b],
            "wq_t": prep(np.asarray(Wq)[hs, :].T),
            "wk_t": prep(np.asarray(Wk)[hs, :].T),
            "wv_t": prep(np.asarray(Wv)[hs, :].T),
            "wo_t": prep(np.asarray(Wo)[:, hs].T),
            "bq_p": np.ascontiguousarray(
                np.asarray(bq, np.float32)[hs].reshape(2, 128).T),
            "bk_p": np.ascontiguousarray(
                np.asarray(bk, np.float32)[hs].reshape(2, 128).T),
            "bv_b": np.ascontiguousarray(
                np.broadcast_to(np.asarray(bv, np.float32)[hs], (128, DPC))),
            "bo_b": np.ascontiguousarray(bo_b),
        })
    return in_maps


_NC = None


def build_nc(loop_n=None):
    nc = bacc.Bacc(
        "TRN2",
        target_bir_lowering=False,
        debug=False,
        enable_asserts=False,
        num_devices=N_CORES,
    )
    ins = {}
    for name in ("xq_t", "xk_t", "xv_t"):
        ins[name] = nc.dram_tensor(
            name, [D_MODEL, S], SB_DT, kind="ExternalInput").ap()
    for name in ("wq_t", "wk_t", "wv_t"):
        ins[name] = nc.dram_tensor(
            name, [D_MODEL, DPC], SB_DT, kind="ExternalInput").ap()
    ins["wo_t"] = nc.dram_tensor(
        "wo_t", [DPC, D_MODEL], SB_DT, kind="ExternalInput").ap()
    ins["bq_p"] = nc.dram_tensor("bq_p", [128, 2], F32, kind="ExternalInput").ap()
    ins["bk_p"] = nc.dram_tensor("bk_p", [128, 2], F32, kind="ExternalInput").ap()
    ins["bv_b"] = nc.dram_tensor("bv_b", [128, DPC], F32, kind="ExternalInput").ap()
    ins["bo_b"] = nc.dram_tensor(
        "bo_b", [128, D_MODEL], F32, kind="ExternalInput").ap()
    out_ap = nc.dram_tensor("out", [S, D_MODEL], OUT_DT, kind="ExternalOutput").ap()
    with tile.TileContext(nc) as tc:
        build_mha(tc, ins, out_ap, loop_n=loop_n)
    nc.compile()
    return nc


def _get_nc():
    global _NC
    if _NC is None:
        _NC = build_nc()
    return _NC


def run_sharded(inputs, trace=False):
    nc = _get_nc()
    in_maps = shard_inputs(**inputs)
    res = bass_utils.run_bass_kernel_spmd(
        nc, in_maps, core_ids=list(range(N_CORES)), trace=trace
    )
    acc = np.zeros((B, S, D_MODEL), np.float64)
    for c in range(N_CORES):
        acc[c // 4] += np.asarray(res.results[c]["out"]).astype(np.float64)
    return acc.astype(np.float32), res


def kernel(**inputs):
    out, _ = run_sharded(inputs, trace=False)
    return out
